# revision 9
# baseline (speedup 1.0000x reference)
"""Trainium2 Bass kernel for MultiHeadAttention (B=2, S=2048, D=512, H=8).

Sharding: 8 cores; core c handles batch b=c//4, query-row quarter qi=c%4
(512 rows), all 8 heads. No collectives: fc + LayerNorm complete locally.

Per-core program (all fp32):
  phase 0: load weights/inputs, PE-transpose (identity matmul) to get
           WqT/WkT/WvT/WfcT, kT, vT, qsT; project qhT/khT [dk, S] and
           vh [S, D]; biases folded in (ACT Identity bias / K=1 ones-matmul).
  per head: scores = qhT_h.T @ khT_h (PE, K=64) -> PSUM; exp fused with
           1/sqrt(D) scale + rowsum on ACT (PSUM->SBUF); normalize with
           DVE tensor_scalar (per-partition 1/rowsum); DMA P out;
           PE-transpose P -> PT; AV: outT_h += vh_chunk.T @ PT_chunk.
  tail:    fc = sum_m outT[m].T @ WfcT[m] + bfc (ones-matmul) + residual,
           LayerNorm via bn_stats/bn_aggr, DMA y out.

_build_program(loop_n=R) wraps the whole body in a hardware For_i loop --
used by test.py to measure steady-state per-iteration device time.
"""

import numpy as np

B, S, D, H, DK = 2, 2048, 512, 8, 64
SQ = 512  # query rows per core
N_CORES = 8
TEMP = float(np.sqrt(D))
EPS = 1e-5

_CACHE = {}


def _build_program(loop_n=None):
    import contextlib

    import concourse.bass as bass
    import concourse.tile as tile
    from concourse import bacc, mybir
    from concourse.masks import make_identity

    F32 = mybir.dt.float32
    AF = mybir.ActivationFunctionType

    nc = bacc.Bacc("TRN2", target_bir_lowering=False, debug=False)

    # --- I/O ---
    qs_d = nc.dram_tensor("qs", [SQ, D], F32, kind="ExternalInput").ap()
    k_d = nc.dram_tensor("k", [S, D], F32, kind="ExternalInput").ap()
    v_d = nc.dram_tensor("v", [S, D], F32, kind="ExternalInput").ap()
    w_d = {
        n: nc.dram_tensor(n, [D, D], F32, kind="ExternalInput").ap()
        for n in ("wq", "wk", "wv", "wfc")
    }
    b_d = {
        n: nc.dram_tensor(n, [D], F32, kind="ExternalInput").ap()
        for n in ("bq", "bk", "bv", "bfc")
    }
    gamma_d = nc.dram_tensor("gamma", [D], F32, kind="ExternalInput").ap()
    beta_d = nc.dram_tensor("beta", [D], F32, kind="ExternalInput").ap()
    attn_d = nc.dram_tensor("attn_out", [H, SQ, S], F32, kind="ExternalOutput").ap()
    y_d = nc.dram_tensor("y_out", [SQ, D], F32, kind="ExternalOutput").ap()

    NSK = S // 128  # 16 sk row-chunks
    ND = D // 128  # 4 feature chunks
    NSQ = SQ // 128  # 4 sq sub-chunks

    with tile.TileContext(nc) as tc, contextlib.ExitStack() as ctx:
        const = ctx.enter_context(tc.tile_pool(name="const", bufs=1))
        persist = ctx.enter_context(tc.tile_pool(name="persist", bufs=1))

        ident = const.tile([128, 128], F32, name="ident")
        make_identity(nc, ident)
        ones1 = const.tile([1, 128], F32, name="ones1")
        nc.vector.memset(ones1, 1.0)
        eps_t = const.tile([128, 1], F32, name="eps_t")
        nc.vector.memset(eps_t, EPS)
        # per-partition bias columns: [128, ND] col m = b[128m:128(m+1)]
        bq_col = const.tile([128, ND], F32, name="bq_col")
        nc.sync.dma_start(out=bq_col, in_=b_d["bq"].rearrange("(m p) -> p m", p=128))
        bk_col = const.tile([128, ND], F32, name="bk_col")
        nc.sync.dma_start(out=bk_col, in_=b_d["bk"].rearrange("(m p) -> p m", p=128))
        # row vectors for ones-matmul bias adds
        bv_row = const.tile([1, D], F32, name="bv_row")
        nc.sync.dma_start(out=bv_row, in_=b_d["bv"].rearrange("(o d) -> o d", o=1))
        bfc_row = const.tile([1, D], F32, name="bfc_row")
        nc.sync.dma_start(out=bfc_row, in_=b_d["bfc"].rearrange("(o d) -> o d", o=1))

        def bcast_rows(ap_1d):
            return bass.AP(
                tensor=ap_1d.tensor, offset=ap_1d.offset,
                ap=[[0, 128]] + [list(d) for d in ap_1d.ap],
            )

        gamma_bc = const.tile([128, D], F32, name="gamma_bc")
        nc.sync.dma_start(out=gamma_bc, in_=bcast_rows(gamma_d))
        beta_bc = const.tile([128, D], F32, name="beta_bc")
        nc.sync.dma_start(out=beta_bc, in_=bcast_rows(beta_d))

        def emit_iteration():
            # per-iteration persistent activations (slot-rotated, bufs=1)
            qs_nat = persist.tile([128, NSQ, D], F32, tag="qs_nat", name="qs_nat")
            nc.sync.dma_start(out=qs_nat, in_=qs_d.rearrange("(t p) d -> p t d", p=128))
            khT = [persist.tile([128, S], F32, tag=f"khT{m}", name=f"khT{m}")
                   for m in range(ND)]
            qhT = [persist.tile([128, SQ], F32, tag=f"qhT{m}", name=f"qhT{m}")
                   for m in range(ND)]
            vh = persist.tile([128, NSK, D], F32, tag="vh", name="vh")
            wfcT = [persist.tile([128, D], F32, tag=f"wfcT{m}", name=f"wfcT{m}")
                    for m in range(ND)]
            outT = [persist.tile([128, SQ], F32, tag=f"outT{m}", name=f"outT{m}")
                    for m in range(ND)]

            # ---------------- phase 0: transposes + projections ----------------
            with tc.tile_pool(name="ph0", bufs=1) as ph0, \
                 tc.tile_pool(name="ph0s", bufs=3) as ph0s, \
                 tc.tile_pool(name="ps0", bufs=2, space="PSUM") as ps0:

                def transpose_128xD_rows(dst_tiles, src_getter, n_row_chunks,
                                         copy_eng):
                    # group-blocked so each source chunk's lifetime is one group
                    for g in range((n_row_chunks + 3) // 4):
                        cnt = min(4, n_row_chunks - 4 * g)
                        srcs = [src_getter(4 * g + u) for u in range(cnt)]
                        for j in range(ND):
                            pt = ps0.tile([128, 512], F32, tag="tr",
                                          name=f"ptr_{j}_{g}")
                            for u in range(cnt):
                                nc.tensor.transpose(
                                    pt[:, 128 * u : 128 * (u + 1)],
                                    srcs[u][:, 128 * j : 128 * (j + 1)],
                                    ident,
                                )
                            if copy_eng == "act":
                                nc.scalar.copy(
                                    dst_tiles[j][:, 512 * g : 512 * g + 128 * cnt],
                                    pt[:, : 128 * cnt],
                                )
                            else:
                                nc.vector.tensor_copy(
                                    dst_tiles[j][:, 512 * g : 512 * g + 128 * cnt],
                                    pt[:, : 128 * cnt],
                                )

                # weights: load natural, transpose
                wT = {}
                for wi, wn in enumerate(("wq", "wk", "wv", "wfc")):
                    w_nat = ph0.tile([128, ND, D], F32, tag=f"wnat{wi % 2}",
                                     name=f"wnat_{wn}")
                    nc.sync.dma_start(
                        out=w_nat, in_=w_d[wn].rearrange("(t p) d -> p t d", p=128)
                    )
                    if wn == "wfc":
                        dst = wfcT
                    else:
                        dst = [ph0.tile([128, D], F32, tag=f"{wn}T{m}",
                                        name=f"{wn}T{m}") for m in range(ND)]
                        wT[wn] = dst
                    transpose_128xD_rows(dst, lambda i: w_nat[:, i, :], ND,
                                         "act" if wi % 2 else "dve")

                # kT / vT, then projections; k and v processed sequentially
                for name, src_d, n_rows in (("k", k_d, NSK), ("v", v_d, NSK)):
                    xT = [ph0.tile([128, S], F32, tag=f"xT{m}", name=f"{name}T{m}")
                          for m in range(ND)]

                    def load_chunk(i, _src=src_d, _name=name):
                        t = ph0s.tile([128, D], F32, tag="ld", bufs=8,
                                      name=f"{_name}n_{i}")
                        nc.sync.dma_start(out=t, in_=_src[128 * i : 128 * (i + 1), :])
                        return t

                    transpose_128xD_rows(xT, load_chunk, n_rows, "dve")

                    if name == "k":
                        for m in range(ND):
                            for n in range(S // 512):
                                pp = ps0.tile([128, 512], F32, tag="proj",
                                              name=f"pk_{m}_{n}")
                                for i in range(ND):
                                    nc.tensor.matmul(
                                        pp,
                                        wT["wk"][i][:, 128 * m : 128 * (m + 1)],
                                        xT[i][:, 512 * n : 512 * (n + 1)],
                                        start=(i == 0),
                                        stop=(i == ND - 1),
                                    )
                                nc.scalar.activation(
                                    khT[m][:, 512 * n : 512 * (n + 1)], pp,
                                    AF.Identity, bias=bk_col[:, m : m + 1],
                                )
                    else:
                        for p in range(NSK):
                            pp = ps0.tile([128, 512], F32, tag="proj",
                                          name=f"pv_{p}")
                            for i in range(ND):
                                nc.tensor.matmul(
                                    pp,
                                    xT[i][:, 128 * p : 128 * (p + 1)],
                                    wT["wv"][i],
                                    start=(i == 0),
                                    stop=False,
                                )
                            nc.tensor.matmul(pp, ones1, bv_row, start=False,
                                             stop=True)
                            nc.vector.tensor_copy(vh[:, p, :], pp)

                # qsT + qhT projection
                qsT = [ph0.tile([128, SQ], F32, tag=f"xT{m}", name=f"qsT{m}")
                       for m in range(ND)]
                transpose_128xD_rows(qsT, lambda i: qs_nat[:, i, :], NSQ, "dve")
                for m in range(ND):
                    pp = ps0.tile([128, 512], F32, tag="proj", name=f"pq_{m}")
                    for i in range(ND):
                        nc.tensor.matmul(
                            pp,
                            wT["wq"][i][:, 128 * m : 128 * (m + 1)],
                            qsT[i],
                            start=(i == 0),
                            stop=(i == ND - 1),
                        )
                    nc.scalar.activation(
                        qhT[m], pp, AF.Identity, bias=bq_col[:, m : m + 1],
                    )

            # ---------------- attention per head ----------------
            with tc.tile_pool(name="pn_pool", bufs=6) as pn_pool, \
                 tc.tile_pool(name="pt_pool", bufs=18) as pt_pool, \
                 tc.tile_pool(name="small", bufs=4) as small, \
                 tc.tile_pool(name="psA", bufs=2, space="PSUM") as psA:

                for h in range(H):
                    hp, hr = h // 2, h % 2
                    qhT_h = qhT[hp][64 * hr : 64 * hr + 64, :]
                    khT_h = khT[hp][64 * hr : 64 * hr + 64, :]

                    P_tiles = []
                    for s in range(NSQ):
                        P_s = pn_pool.tile([128, S], F32, tag="pn",
                                           name=f"P_{h}_{s}")
                        rs4 = small.tile([128, 4], F32, tag="rs4",
                                         name=f"rs4_{h}_{s}")
                        for n in range(S // 512):
                            psc = psA.tile([128, 512], F32, tag="sc", bufs=3,
                                           name=f"sc_{h}_{s}_{n}")
                            nc.tensor.matmul(
                                psc,
                                qhT_h[:, 128 * s : 128 * (s + 1)],
                                khT_h[:, 512 * n : 512 * (n + 1)],
                                start=True, stop=True,
                            )
                            nc.scalar.activation(
                                P_s[:, 512 * n : 512 * (n + 1)], psc, AF.Exp,
                                scale=1.0 / TEMP, accum_out=rs4[:, n : n + 1],
                            )
                        rs1 = small.tile([128, 1], F32, tag="rs1",
                                         name=f"rs1_{h}_{s}")
                        nc.vector.reduce_sum(rs1, rs4, axis=mybir.AxisListType.X)
                        inv = small.tile([128, 1], F32, tag="inv",
                                         name=f"inv_{h}_{s}")
                        nc.vector.reciprocal(inv, rs1)
                        nc.vector.tensor_scalar_mul(P_s, P_s, inv)
                        nc.sync.dma_start(
                            out=attn_d[h, 128 * s : 128 * (s + 1), :], in_=P_s
                        )
                        P_tiles.append(P_s)

                    # transpose P -> PT tiles [128 sk, SQ]
                    PT_tiles = []
                    for skc in range(NSK):
                        ptp = psA.tile([128, 512], F32, tag="pt", bufs=2,
                                       name=f"ptp_{h}_{skc}")
                        for s in range(NSQ):
                            nc.tensor.transpose(
                                ptp[:, 128 * s : 128 * (s + 1)],
                                P_tiles[s][:, 128 * skc : 128 * (skc + 1)],
                                ident,
                            )
                        PT_k = pt_pool.tile([128, SQ], F32, tag="ptsb",
                                            name=f"PT_{h}_{skc}")
                        if skc % 2 == 0:
                            nc.vector.tensor_copy(PT_k, ptp)
                        else:
                            nc.scalar.copy(PT_k, ptp)
                        PT_tiles.append(PT_k)

                    # AV: outT_h [64, SQ] accumulated over sk chunks
                    pav = psA.tile([64, 512], F32, tag="av", bufs=2,
                                   name=f"pav_{h}")
                    for skc in range(NSK):
                        nc.tensor.matmul(
                            pav,
                            vh[:, skc, 64 * h : 64 * h + 64],
                            PT_tiles[skc],
                            start=(skc == 0),
                            stop=(skc == NSK - 1),
                        )
                    nc.vector.tensor_copy(outT[hp][64 * hr : 64 * hr + 64, :], pav)

            # ---------------- fc + residual + LayerNorm ----------------
            with tc.tile_pool(name="tail", bufs=4) as tail, \
                 tc.tile_pool(name="psY", bufs=2, space="PSUM") as psY:
                for s in range(NSQ):
                    py = psY.tile([128, 512], F32, tag="y", name=f"py_{s}")
                    for m in range(ND):
                        nc.tensor.matmul(
                            py,
                            outT[m][:, 128 * s : 128 * (s + 1)],
                            wfcT[m],
                            start=(m == 0),
                            stop=False,
                        )
                    nc.tensor.matmul(py, ones1, bfc_row, start=False, stop=True)
                    x_sb = tail.tile([128, D], F32, tag="x", name=f"x_{s}")
                    nc.vector.tensor_add(x_sb, py, qs_nat[:, s, :])
                    stats = tail.tile([128, 6], F32, tag="st", name=f"st_{s}")
                    nc.vector.bn_stats(out=stats, in_=x_sb)
                    mv = tail.tile([128, 2], F32, tag="mv", name=f"mv_{s}")
                    nc.vector.bn_aggr(out=mv, in_=stats)
                    rstd = tail.tile([128, 1], F32, tag="rstd", name=f"rstd_{s}")
                    nc.scalar.activation(rstd, mv[:, 1:2], AF.Sqrt, bias=eps_t)
                    nc.vector.reciprocal(rstd, rstd)
                    xn = tail.tile([128, D], F32, tag="xn", name=f"xn_{s}")
                    nc.vector.tensor_scalar(
                        xn, x_sb, mv[:, 0:1], rstd,
                        op0=mybir.AluOpType.subtract, op1=mybir.AluOpType.mult,
                    )
                    nc.vector.tensor_mul(xn, xn, gamma_bc)
                    nc.vector.tensor_add(xn, xn, beta_bc)
                    nc.sync.dma_start(out=y_d[128 * s : 128 * (s + 1), :], in_=xn)

        if loop_n:
            with tc.For_i(0, loop_n, 1,
                          hint_engines=(mybir.EngineType.PE,
                                        mybir.EngineType.Activation,
                                        mybir.EngineType.DVE)):
                emit_iteration()
        else:
            emit_iteration()

    nc.compile()
    return nc


def _make_runner(nc):
    """Cached jitted SPMD runner for a compiled Bass program (mirrors
    concourse.bass2jax.run_bass_via_pjrt's multi-core branch)."""
    import jax
    from jax.experimental.shard_map import shard_map
    from jax.sharding import Mesh, PartitionSpec
    from concourse import bass2jax, mybir as _mybir

    bass2jax.install_neuronx_cc_hook()

    partition_name = nc.partition_id_tensor.name if nc.partition_id_tensor else None
    in_names, out_names, out_avals, zero_outs = [], [], [], []
    for alloc in nc.m.functions[0].allocations:
        if not isinstance(alloc, _mybir.MemoryLocationSet):
            continue
        name = alloc.memorylocations[0].name
        if alloc.kind == "ExternalInput":
            if name != partition_name:
                in_names.append(name)
        elif alloc.kind == "ExternalOutput":
            shape = tuple(alloc.tensor_shape)
            dtype = _mybir.dt.np(alloc.dtype)
            out_names.append(name)
            out_avals.append(jax.core.ShapedArray(shape, dtype))
            zero_outs.append(np.zeros(shape, dtype))
    n_params = len(in_names)
    n_outs = len(out_avals)
    all_in_names = in_names + out_names + (
        [partition_name] if partition_name else []
    )
    donate = tuple(range(n_params, n_params + n_outs))

    def _body(*args):
        operands = list(args)
        if partition_name is not None:
            operands.append(bass2jax.partition_id_tensor())
        outs = bass2jax._bass_exec_p.bind(
            *operands,
            out_avals=tuple(out_avals),
            in_names=tuple(all_in_names),
            out_names=tuple(out_names),
            lowering_input_output_aliases=(),
            sim_require_finite=True,
            sim_require_nnan=True,
            nc=nc,
        )
        return tuple(outs)

    devices = jax.devices()[:N_CORES]
    mesh = Mesh(np.asarray(devices), ("core",))
    in_specs = (PartitionSpec("core"),) * (n_params + n_outs)
    out_specs = (PartitionSpec("core"),) * n_outs
    sharded = jax.jit(
        shard_map(_body, mesh=mesh, in_specs=in_specs, out_specs=out_specs,
                  check_rep=False),
        donate_argnums=donate,
        keep_unused=True,
    )

    def run(in_maps):
        per_core = [[np.asarray(m[n]) for n in in_names] for m in in_maps]
        concat_in = [
            np.concatenate([per_core[c][i] for c in range(N_CORES)], axis=0)
            for i in range(n_params)
        ]
        concat_zeros = [
            np.zeros((N_CORES * z.shape[0], *z.shape[1:]), z.dtype)
            for z in zero_outs
        ]
        out_arrs = sharded(*concat_in, *concat_zeros)
        return [
            {
                n: np.asarray(out_arrs[i]).reshape(N_CORES, *out_avals[i].shape)[c]
                for i, n in enumerate(out_names)
            }
            for c in range(N_CORES)
        ]

    return run


def _make_bench_fn(nc, in_maps):
    """Timing-only runner: inputs (and dummy zero outputs) are device_put
    once; no donation, outputs never fetched. Per-call cost = dispatch +
    device execution."""
    import jax
    from jax.experimental.shard_map import shard_map
    from jax.sharding import Mesh, NamedSharding, PartitionSpec
    from concourse import bass2jax, mybir as _mybir

    bass2jax.install_neuronx_cc_hook()

    partition_name = nc.partition_id_tensor.name if nc.partition_id_tensor else None
    in_names, out_names, out_avals, zero_outs = [], [], [], []
    for alloc in nc.m.functions[0].allocations:
        if not isinstance(alloc, _mybir.MemoryLocationSet):
            continue
        name = alloc.memorylocations[0].name
        if alloc.kind == "ExternalInput":
            if name != partition_name:
                in_names.append(name)
        elif alloc.kind == "ExternalOutput":
            shape = tuple(alloc.tensor_shape)
            dtype = _mybir.dt.np(alloc.dtype)
            out_names.append(name)
            out_avals.append(jax.core.ShapedArray(shape, dtype))
            zero_outs.append(np.zeros(shape, dtype))
    n_params = len(in_names)
    all_in_names = in_names + out_names + (
        [partition_name] if partition_name else []
    )

    def _body(*args):
        operands = list(args)
        if partition_name is not None:
            operands.append(bass2jax.partition_id_tensor())
        outs = bass2jax._bass_exec_p.bind(
            *operands,
            out_avals=tuple(out_avals),
            in_names=tuple(all_in_names),
            out_names=tuple(out_names),
            lowering_input_output_aliases=(),
            sim_require_finite=True,
            sim_require_nnan=True,
            nc=nc,
        )
        return tuple(outs)

    devices = jax.devices()[:N_CORES]
    mesh = Mesh(np.asarray(devices), ("core",))
    nsh = NamedSharding(mesh, PartitionSpec("core"))
    in_specs = (PartitionSpec("core"),) * (n_params + len(out_avals))
    out_specs = (PartitionSpec("core"),) * len(out_avals)
    sharded = jax.jit(
        shard_map(_body, mesh=mesh, in_specs=in_specs, out_specs=out_specs,
                  check_rep=False),
        keep_unused=True,
    )

    per_core = [[np.asarray(m[n]) for n in in_names] for m in in_maps]
    dev_args = [
        jax.device_put(
            np.concatenate([per_core[c][i] for c in range(N_CORES)], axis=0), nsh
        )
        for i in range(n_params)
    ] + [
        jax.device_put(np.zeros((N_CORES * z.shape[0], *z.shape[1:]), z.dtype), nsh)
        for z in zero_outs
    ]

    def call_once():
        import time as _t
        t0 = _t.time()
        out = sharded(*dev_args)
        jax.block_until_ready(out)
        return _t.time() - t0

    return call_once


def _shard_inputs(q, k, v, Wq, bq, Wk, bk, Wv, bv, Wfc, bfc, ln_gamma, ln_beta):
    q = np.ascontiguousarray(np.asarray(q, dtype=np.float32))
    k = np.ascontiguousarray(np.asarray(k, dtype=np.float32))
    v = np.ascontiguousarray(np.asarray(v, dtype=np.float32))
    common = {
        "wq": np.ascontiguousarray(np.asarray(Wq, np.float32)),
        "wk": np.ascontiguousarray(np.asarray(Wk, np.float32)),
        "wv": np.ascontiguousarray(np.asarray(Wv, np.float32)),
        "wfc": np.ascontiguousarray(np.asarray(Wfc, np.float32)),
        "bq": np.ascontiguousarray(np.asarray(bq, np.float32)),
        "bk": np.ascontiguousarray(np.asarray(bk, np.float32)),
        "bv": np.ascontiguousarray(np.asarray(bv, np.float32)),
        "bfc": np.ascontiguousarray(np.asarray(bfc, np.float32)),
        "gamma": np.ascontiguousarray(np.asarray(ln_gamma, np.float32)),
        "beta": np.ascontiguousarray(np.asarray(ln_beta, np.float32)),
    }
    in_maps = []
    for c in range(N_CORES):
        b, qi = c // 4, c % 4
        rows = slice(SQ * qi, SQ * (qi + 1))
        in_maps.append(
            {
                "qs": np.ascontiguousarray(q[b, rows, :]),
                "k": np.ascontiguousarray(k[b]),
                "v": np.ascontiguousarray(v[b]),
                **common,
            }
        )
    return in_maps


def kernel(q, k, v, Wq, bq, Wk, bk, Wv, bv, Wfc, bfc, ln_gamma, ln_beta):
    if "run" not in _CACHE:
        _CACHE["run"] = _make_runner(_build_program())
    in_maps = _shard_inputs(q, k, v, Wq, bq, Wk, bk, Wv, bv, Wfc, bfc,
                            ln_gamma, ln_beta)
    results = _CACHE["run"](in_maps)
    attn_flat = np.empty((H * B, S, S), np.float32)
    y = np.empty((B, S, D), np.float32)
    for c in range(N_CORES):
        b, qi = c // 4, c % 4
        rows = slice(SQ * qi, SQ * (qi + 1))
        for h in range(H):
            attn_flat[h * B + b, rows, :] = results[c]["attn_out"][h]
        y[b, rows, :] = results[c]["y_out"]
    return (y, attn_flat)


# revision 18
# speedup vs baseline: 1.7438x; 1.7438x over previous
"""Trainium2 Bass kernel for MultiHeadAttention (B=2, S=2048, D=512, H=8).

Sharding: 8 cores; core c handles batch b=c//4, query-row quarter qi=c%4
(512 rows), all 8 heads. No collectives: fc + LayerNorm complete locally.

Per-core program (all fp32):
  phase 0: load weights/inputs, PE-transpose (identity matmul) to get
           WqT/WkT/WvT/WfcT, kT, vT, qsT; project qhT/khT [dk, S] and
           vh [S, D]; biases folded in (ACT Identity bias / K=1 ones-matmul).
  per head: scores = qhT_h.T @ khT_h (PE, K=64) -> PSUM; exp fused with
           1/sqrt(D) scale + rowsum on ACT (PSUM->SBUF); normalize with
           DVE tensor_scalar (per-partition 1/rowsum); DMA P out;
           PE-transpose P -> PT; AV: outT_h += vh_chunk.T @ PT_chunk.
  tail:    fc = sum_m outT[m].T @ WfcT[m] + bfc (ones-matmul) + residual,
           LayerNorm via bn_stats/bn_aggr, DMA y out.

_build_program(loop_n=R) wraps the whole body in a hardware For_i loop --
used by test.py to measure steady-state per-iteration device time.
"""

import numpy as np

B, S, D, H, DK = 2, 2048, 512, 8, 64
SQ = 512  # query rows per core
N_CORES = 8
TEMP = float(np.sqrt(D))
EPS = 1e-5

_CACHE = {}


def _build_program(loop_n=None, r32=True):
    import contextlib

    import concourse.bass as bass
    import concourse.tile as tile
    from concourse import bacc, mybir
    from concourse.masks import make_identity

    F32 = mybir.dt.float32
    AF = mybir.ActivationFunctionType

    # float32r: single-pass PE fp32 (1 cycle/row at N>=256 vs 4 cycles/row
    # for exact fp32). Tiles feeding matmuls are declared float32r so their
    # producing copy/activation rounds on write (BIR verifier requirement).
    MM = mybir.dt.float32r if r32 else F32

    def R(ap):
        return ap

    nc = bacc.Bacc("TRN2", target_bir_lowering=False, debug=False)

    # --- I/O ---
    qs_d = nc.dram_tensor("qs", [SQ, D], F32, kind="ExternalInput").ap()
    k_d = nc.dram_tensor("k", [S, D], F32, kind="ExternalInput").ap()
    v_d = nc.dram_tensor("v", [S, D], F32, kind="ExternalInput").ap()
    w_d = {
        n: nc.dram_tensor(n, [D, D], F32, kind="ExternalInput").ap()
        for n in ("wq", "wk", "wv", "wfc")
    }
    b_d = {
        n: nc.dram_tensor(n, [D], F32, kind="ExternalInput").ap()
        for n in ("bq", "bk", "bv", "bfc")
    }
    gamma_d = nc.dram_tensor("gamma", [D], F32, kind="ExternalInput").ap()
    beta_d = nc.dram_tensor("beta", [D], F32, kind="ExternalInput").ap()
    # transposed per-head layout [sk, sq]; host gather transposes back
    attn_d = nc.dram_tensor("attn_out", [H, S, SQ], F32, kind="ExternalOutput").ap()
    y_d = nc.dram_tensor("y_out", [SQ, D], F32, kind="ExternalOutput").ap()

    NSK = S // 128  # 16 sk row-chunks
    ND = D // 128  # 4 feature chunks
    NSQ = SQ // 128  # 4 sq sub-chunks

    with tile.TileContext(nc) as tc, contextlib.ExitStack() as ctx:
        const = ctx.enter_context(tc.tile_pool(name="const", bufs=1))
        persist = ctx.enter_context(tc.tile_pool(name="persist", bufs=1))

        ident = const.tile([128, 128], F32, name="ident")
        make_identity(nc, ident)
        ones_f32 = const.tile([128, 128], F32, name="ones_f32")
        nc.vector.memset(ones_f32, 1.0)
        ones1 = const.tile([1, 128], MM, name="ones1")
        nc.vector.tensor_copy(ones1, ones_f32[0:1, :])
        ones_col = const.tile([128, 1], MM, name="ones_col")
        nc.vector.tensor_copy(ones_col, ones_f32[:, 0:1])
        eps_t = const.tile([128, 1], F32, name="eps_t")
        nc.vector.memset(eps_t, EPS)
        # per-partition bias columns: [128, ND] col m = b[128m:128(m+1)]
        bq_col = const.tile([128, ND], F32, name="bq_col")
        nc.sync.dma_start(out=bq_col, in_=b_d["bq"].rearrange("(m p) -> p m", p=128))
        bk_col = const.tile([128, ND], F32, name="bk_col")
        nc.sync.dma_start(out=bk_col, in_=b_d["bk"].rearrange("(m p) -> p m", p=128))
        # row vectors for ones-matmul bias adds
        bv_row = const.tile([1, D], MM, name="bv_row")
        nc.sync.dma_start(out=bv_row, in_=b_d["bv"].rearrange("(o d) -> o d", o=1).bitcast(MM))
        bfc_row = const.tile([1, D], MM, name="bfc_row")
        nc.sync.dma_start(out=bfc_row, in_=b_d["bfc"].rearrange("(o d) -> o d", o=1).bitcast(MM))

        def bcast_rows(ap_1d):
            return bass.AP(
                tensor=ap_1d.tensor, offset=ap_1d.offset,
                ap=[[0, 128]] + [list(d) for d in ap_1d.ap],
            )

        gamma_bc = const.tile([128, D], F32, name="gamma_bc")
        nc.sync.dma_start(out=gamma_bc, in_=bcast_rows(gamma_d))
        beta_bc = const.tile([128, D], F32, name="beta_bc")
        nc.sync.dma_start(out=beta_bc, in_=bcast_rows(beta_d))

        def emit_iteration():
            # per-iteration persistent activations (slot-rotated, bufs=1)
            qs_nat = persist.tile([128, NSQ, D], F32, tag="qs_nat", name="qs_nat")
            nc.sync.dma_start(out=qs_nat, in_=qs_d.rearrange("(t p) d -> p t d", p=128))
            khT = [persist.tile([128, S], MM, tag=f"khT{m}", name=f"khT{m}")
                   for m in range(ND)]
            qhT = [persist.tile([128, SQ], MM, tag=f"qhT{m}", name=f"qhT{m}")
                   for m in range(ND)]
            vh = persist.tile([128, NSK, D], MM, tag="vh", name="vh")
            wfcT = [persist.tile([128, D], MM, tag=f"wfcT{m}", name=f"wfcT{m}")
                    for m in range(ND)]
            outT = [persist.tile([128, SQ], MM, tag=f"outT{m}", name=f"outT{m}")
                    for m in range(ND)]

            # ---------------- phase 0: transposes + projections ----------------
            with tc.tile_pool(name="ph0", bufs=1) as ph0, \
                 tc.tile_pool(name="ph0s", bufs=3) as ph0s, \
                 tc.tile_pool(name="ps0", bufs=2, space="PSUM") as ps0:

                def transpose_128xD_rows(dst_tiles, src_getter, n_row_chunks,
                                         copy_eng):
                    # group-blocked so each source chunk's lifetime is one group
                    for g in range((n_row_chunks + 3) // 4):
                        cnt = min(4, n_row_chunks - 4 * g)
                        srcs = [src_getter(4 * g + u) for u in range(cnt)]
                        for j in range(ND):
                            pt = ps0.tile([128, 512], F32, tag="tr",
                                          name=f"ptr_{j}_{g}")
                            for u in range(cnt):
                                nc.tensor.transpose(
                                    pt[:, 128 * u : 128 * (u + 1)],
                                    srcs[u][:, 128 * j : 128 * (j + 1)],
                                    ident,
                                )
                            if copy_eng == "act":
                                nc.scalar.copy(
                                    dst_tiles[j][:, 512 * g : 512 * g + 128 * cnt],
                                    pt[:, : 128 * cnt],
                                )
                            else:
                                nc.vector.tensor_copy(
                                    dst_tiles[j][:, 512 * g : 512 * g + 128 * cnt],
                                    pt[:, : 128 * cnt],
                                )

                # weights: load natural, transpose
                wT = {}
                for wi, wn in enumerate(("wq", "wk", "wv", "wfc")):
                    w_nat = ph0.tile([128, ND, D], F32, tag=f"wnat{wi % 2}",
                                     name=f"wnat_{wn}")
                    nc.sync.dma_start(
                        out=w_nat, in_=w_d[wn].rearrange("(t p) d -> p t d", p=128)
                    )
                    if wn == "wfc":
                        dst = wfcT
                    else:
                        dst = [ph0.tile([128, D], MM, tag=f"{wn}T{m}",
                                        name=f"{wn}T{m}") for m in range(ND)]
                        wT[wn] = dst
                    transpose_128xD_rows(dst, lambda i: w_nat[:, i, :], ND,
                                         "act" if wi % 2 else "dve")

                # kT / vT, then projections; k and v processed sequentially
                for name, src_d, n_rows in (("k", k_d, NSK), ("v", v_d, NSK)):
                    xT = [ph0.tile([128, S], MM, tag=f"xT{m}", name=f"{name}T{m}")
                          for m in range(ND)]

                    def load_chunk(i, _src=src_d, _name=name):
                        t = ph0s.tile([128, D], F32, tag="ld", bufs=8,
                                      name=f"{_name}n_{i}")
                        nc.sync.dma_start(out=t, in_=_src[128 * i : 128 * (i + 1), :])
                        return t

                    transpose_128xD_rows(xT, load_chunk, n_rows, "dve")

                    if name == "k":
                        for m in range(ND):
                            for n in range(S // 512):
                                pp = ps0.tile([128, 512], F32, tag="proj",
                                              name=f"pk_{m}_{n}")
                                for i in range(ND):
                                    nc.tensor.matmul(
                                        pp,
                                        R(wT["wk"][i][:, 128 * m : 128 * (m + 1)]),
                                        R(xT[i][:, 512 * n : 512 * (n + 1)]),
                                        start=(i == 0),
                                        stop=(i == ND - 1),
                                    )
                                nc.scalar.activation(
                                    khT[m][:, 512 * n : 512 * (n + 1)], pp,
                                    AF.Identity, bias=bk_col[:, m : m + 1],
                                )
                    else:
                        for p in range(NSK):
                            pp = ps0.tile([128, 512], F32, tag="proj",
                                          name=f"pv_{p}")
                            for i in range(ND):
                                nc.tensor.matmul(
                                    pp,
                                    R(xT[i][:, 128 * p : 128 * (p + 1)]),
                                    R(wT["wv"][i]),
                                    start=(i == 0),
                                    stop=False,
                                )
                            nc.tensor.matmul(pp, R(ones1), R(bv_row), start=False,
                                             stop=True)
                            nc.vector.tensor_copy(vh[:, p, :], pp)

                # qsT + qhT projection
                qsT = [ph0.tile([128, SQ], MM, tag=f"xT{m}", name=f"qsT{m}")
                       for m in range(ND)]
                transpose_128xD_rows(qsT, lambda i: qs_nat[:, i, :], NSQ, "dve")
                for m in range(ND):
                    pp = ps0.tile([128, 512], F32, tag="proj", name=f"pq_{m}")
                    for i in range(ND):
                        nc.tensor.matmul(
                            pp,
                            R(wT["wq"][i][:, 128 * m : 128 * (m + 1)]),
                            R(qsT[i]),
                            start=(i == 0),
                            stop=(i == ND - 1),
                        )
                    nc.scalar.activation(
                        qhT[m], pp, AF.Identity, bias=bq_col[:, m : m + 1],
                    )

            # ---------------- attention per head (transposed flow) ----------
            # sT[sk, sq] = khT_h.T @ qhT_h; exp fused in the PSUM->SBUF copy;
            # colsum via ones-matmul on PE; normalize P~T in place; write
            # attn transposed; AV consumes P~T chunks directly.
            with tc.tile_pool(name="expt_pool", bufs=2) as expt_pool, \
                 tc.tile_pool(name="small", bufs=4) as small, \
                 tc.tile_pool(name="psA", bufs=2, space="PSUM") as psA:

                for h in range(H):
                    hp, hr = h // 2, h % 2
                    qhT_h = qhT[hp][64 * hr : 64 * hr + 64, :]
                    khT_h = khT[hp][64 * hr : 64 * hr + 64, :]

                    expT = expt_pool.tile([128, NSK, SQ], MM, tag="expt",
                                          name=f"expT_{h}")
                    pcs = psA.tile([1, SQ], F32, tag="cs", bufs=2,
                                   name=f"pcs_{h}")
                    for skc in range(NSK):
                        psc = psA.tile([128, SQ], F32, tag="sc", bufs=3,
                                       name=f"sc_{h}_{skc}")
                        nc.tensor.matmul(
                            psc,
                            R(khT_h[:, 128 * skc : 128 * (skc + 1)]),
                            R(qhT_h),
                            start=True, stop=True,
                        )
                        nc.scalar.activation(
                            expT[:, skc, :], psc, AF.Exp, scale=1.0 / TEMP,
                        )
                        nc.tensor.matmul(
                            pcs, R(ones_col), R(expT[:, skc, :]),
                            start=(skc == 0), stop=(skc == NSK - 1),
                        )
                    rs_sb = small.tile([1, SQ], F32, tag="rs", name=f"rs_{h}")
                    nc.vector.tensor_copy(rs_sb, pcs)
                    inv_r = small.tile([1, SQ], F32, tag="invr", name=f"inv_{h}")
                    nc.vector.reciprocal(inv_r, rs_sb)
                    inv_bc = small.tile([128, SQ], F32, tag="invbc",
                                        name=f"invbc_{h}")
                    nc.gpsimd.partition_broadcast(inv_bc, inv_r)
                    # normalize in place; inv broadcast along the skc dim
                    inv_bc3 = bass.AP(
                        tensor=inv_bc.tensor, offset=inv_bc.offset,
                        ap=[list(inv_bc.ap[0]), [0, NSK], list(inv_bc.ap[1])],
                    )
                    nc.vector.tensor_mul(expT, expT, inv_bc3)
                    nc.sync.dma_start(
                        out=attn_d[h].rearrange("(t p) q -> p t q", p=128),
                        in_=expT.bitcast(F32),
                    )

                    # AV: outT_h [64, SQ] accumulated over sk chunks
                    pav = psA.tile([64, 512], F32, tag="av", bufs=2,
                                   name=f"pav_{h}")
                    for skc in range(NSK):
                        nc.tensor.matmul(
                            pav,
                            R(vh[:, skc, 64 * h : 64 * h + 64]),
                            R(expT[:, skc, :]),
                            start=(skc == 0),
                            stop=(skc == NSK - 1),
                        )
                    nc.vector.tensor_copy(outT[hp][64 * hr : 64 * hr + 64, :], pav)

            # ---------------- fc + residual + LayerNorm ----------------
            with tc.tile_pool(name="tail", bufs=4) as tail, \
                 tc.tile_pool(name="psY", bufs=2, space="PSUM") as psY:
                for s in range(NSQ):
                    py = psY.tile([128, 512], F32, tag="y", name=f"py_{s}")
                    for m in range(ND):
                        nc.tensor.matmul(
                            py,
                            R(outT[m][:, 128 * s : 128 * (s + 1)]),
                            R(wfcT[m]),
                            start=(m == 0),
                            stop=False,
                        )
                    nc.tensor.matmul(py, R(ones1), R(bfc_row), start=False,
                                     stop=True)
                    x_sb = tail.tile([128, D], F32, tag="x", name=f"x_{s}")
                    nc.vector.tensor_add(x_sb, py, qs_nat[:, s, :])
                    stats = tail.tile([128, 6], F32, tag="st", name=f"st_{s}")
                    nc.vector.bn_stats(out=stats, in_=x_sb)
                    mv = tail.tile([128, 2], F32, tag="mv", name=f"mv_{s}")
                    nc.vector.bn_aggr(out=mv, in_=stats)
                    rstd = tail.tile([128, 1], F32, tag="rstd", name=f"rstd_{s}")
                    nc.scalar.activation(rstd, mv[:, 1:2], AF.Sqrt, bias=eps_t)
                    nc.vector.reciprocal(rstd, rstd)
                    xn = tail.tile([128, D], F32, tag="xn", name=f"xn_{s}")
                    nc.vector.tensor_scalar(
                        xn, x_sb, mv[:, 0:1], rstd,
                        op0=mybir.AluOpType.subtract, op1=mybir.AluOpType.mult,
                    )
                    nc.vector.tensor_mul(xn, xn, gamma_bc)
                    nc.vector.tensor_add(xn, xn, beta_bc)
                    nc.sync.dma_start(out=y_d[128 * s : 128 * (s + 1), :], in_=xn)

        if loop_n:
            with tc.For_i(0, loop_n, 1,
                          hint_engines=(mybir.EngineType.PE,
                                        mybir.EngineType.Activation,
                                        mybir.EngineType.DVE)):
                emit_iteration()
        else:
            emit_iteration()

    nc.compile()
    return nc


def _make_runner(nc):
    """Cached jitted SPMD runner for a compiled Bass program (mirrors
    concourse.bass2jax.run_bass_via_pjrt's multi-core branch)."""
    import jax
    from jax.experimental.shard_map import shard_map
    from jax.sharding import Mesh, PartitionSpec
    from concourse import bass2jax, mybir as _mybir

    bass2jax.install_neuronx_cc_hook()

    partition_name = nc.partition_id_tensor.name if nc.partition_id_tensor else None
    in_names, out_names, out_avals, zero_outs = [], [], [], []
    for alloc in nc.m.functions[0].allocations:
        if not isinstance(alloc, _mybir.MemoryLocationSet):
            continue
        name = alloc.memorylocations[0].name
        if alloc.kind == "ExternalInput":
            if name != partition_name:
                in_names.append(name)
        elif alloc.kind == "ExternalOutput":
            shape = tuple(alloc.tensor_shape)
            dtype = _mybir.dt.np(alloc.dtype)
            out_names.append(name)
            out_avals.append(jax.core.ShapedArray(shape, dtype))
            zero_outs.append(np.zeros(shape, dtype))
    n_params = len(in_names)
    n_outs = len(out_avals)
    all_in_names = in_names + out_names + (
        [partition_name] if partition_name else []
    )
    donate = tuple(range(n_params, n_params + n_outs))

    def _body(*args):
        operands = list(args)
        if partition_name is not None:
            operands.append(bass2jax.partition_id_tensor())
        outs = bass2jax._bass_exec_p.bind(
            *operands,
            out_avals=tuple(out_avals),
            in_names=tuple(all_in_names),
            out_names=tuple(out_names),
            lowering_input_output_aliases=(),
            sim_require_finite=True,
            sim_require_nnan=True,
            nc=nc,
        )
        return tuple(outs)

    devices = jax.devices()[:N_CORES]
    mesh = Mesh(np.asarray(devices), ("core",))
    in_specs = (PartitionSpec("core"),) * (n_params + n_outs)
    out_specs = (PartitionSpec("core"),) * n_outs
    sharded = jax.jit(
        shard_map(_body, mesh=mesh, in_specs=in_specs, out_specs=out_specs,
                  check_rep=False),
        donate_argnums=donate,
        keep_unused=True,
    )

    def run(in_maps):
        per_core = [[np.asarray(m[n]) for n in in_names] for m in in_maps]
        concat_in = [
            np.concatenate([per_core[c][i] for c in range(N_CORES)], axis=0)
            for i in range(n_params)
        ]
        concat_zeros = [
            np.zeros((N_CORES * z.shape[0], *z.shape[1:]), z.dtype)
            for z in zero_outs
        ]
        out_arrs = sharded(*concat_in, *concat_zeros)
        return [
            {
                n: np.asarray(out_arrs[i]).reshape(N_CORES, *out_avals[i].shape)[c]
                for i, n in enumerate(out_names)
            }
            for c in range(N_CORES)
        ]

    return run


def _make_bench_fn(nc, in_maps):
    """Timing-only runner: inputs (and dummy zero outputs) are device_put
    once; no donation, outputs never fetched. Per-call cost = dispatch +
    device execution."""
    import jax
    from jax.experimental.shard_map import shard_map
    from jax.sharding import Mesh, NamedSharding, PartitionSpec
    from concourse import bass2jax, mybir as _mybir

    bass2jax.install_neuronx_cc_hook()

    partition_name = nc.partition_id_tensor.name if nc.partition_id_tensor else None
    in_names, out_names, out_avals, zero_outs = [], [], [], []
    for alloc in nc.m.functions[0].allocations:
        if not isinstance(alloc, _mybir.MemoryLocationSet):
            continue
        name = alloc.memorylocations[0].name
        if alloc.kind == "ExternalInput":
            if name != partition_name:
                in_names.append(name)
        elif alloc.kind == "ExternalOutput":
            shape = tuple(alloc.tensor_shape)
            dtype = _mybir.dt.np(alloc.dtype)
            out_names.append(name)
            out_avals.append(jax.core.ShapedArray(shape, dtype))
            zero_outs.append(np.zeros(shape, dtype))
    n_params = len(in_names)
    all_in_names = in_names + out_names + (
        [partition_name] if partition_name else []
    )

    def _body(*args):
        operands = list(args)
        if partition_name is not None:
            operands.append(bass2jax.partition_id_tensor())
        outs = bass2jax._bass_exec_p.bind(
            *operands,
            out_avals=tuple(out_avals),
            in_names=tuple(all_in_names),
            out_names=tuple(out_names),
            lowering_input_output_aliases=(),
            sim_require_finite=True,
            sim_require_nnan=True,
            nc=nc,
        )
        return tuple(outs)

    devices = jax.devices()[:N_CORES]
    mesh = Mesh(np.asarray(devices), ("core",))
    nsh = NamedSharding(mesh, PartitionSpec("core"))
    in_specs = (PartitionSpec("core"),) * (n_params + len(out_avals))
    out_specs = (PartitionSpec("core"),) * len(out_avals)
    sharded = jax.jit(
        shard_map(_body, mesh=mesh, in_specs=in_specs, out_specs=out_specs,
                  check_rep=False),
        keep_unused=True,
    )

    per_core = [[np.asarray(m[n]) for n in in_names] for m in in_maps]
    dev_args = [
        jax.device_put(
            np.concatenate([per_core[c][i] for c in range(N_CORES)], axis=0), nsh
        )
        for i in range(n_params)
    ] + [
        jax.device_put(np.zeros((N_CORES * z.shape[0], *z.shape[1:]), z.dtype), nsh)
        for z in zero_outs
    ]

    def call_once():
        import time as _t
        t0 = _t.time()
        out = sharded(*dev_args)
        jax.block_until_ready(out)
        return _t.time() - t0

    return call_once


def _shard_inputs(q, k, v, Wq, bq, Wk, bk, Wv, bv, Wfc, bfc, ln_gamma, ln_beta):
    q = np.ascontiguousarray(np.asarray(q, dtype=np.float32))
    k = np.ascontiguousarray(np.asarray(k, dtype=np.float32))
    v = np.ascontiguousarray(np.asarray(v, dtype=np.float32))
    common = {
        "wq": np.ascontiguousarray(np.asarray(Wq, np.float32)),
        "wk": np.ascontiguousarray(np.asarray(Wk, np.float32)),
        "wv": np.ascontiguousarray(np.asarray(Wv, np.float32)),
        "wfc": np.ascontiguousarray(np.asarray(Wfc, np.float32)),
        "bq": np.ascontiguousarray(np.asarray(bq, np.float32)),
        "bk": np.ascontiguousarray(np.asarray(bk, np.float32)),
        "bv": np.ascontiguousarray(np.asarray(bv, np.float32)),
        "bfc": np.ascontiguousarray(np.asarray(bfc, np.float32)),
        "gamma": np.ascontiguousarray(np.asarray(ln_gamma, np.float32)),
        "beta": np.ascontiguousarray(np.asarray(ln_beta, np.float32)),
    }
    in_maps = []
    for c in range(N_CORES):
        b, qi = c // 4, c % 4
        rows = slice(SQ * qi, SQ * (qi + 1))
        in_maps.append(
            {
                "qs": np.ascontiguousarray(q[b, rows, :]),
                "k": np.ascontiguousarray(k[b]),
                "v": np.ascontiguousarray(v[b]),
                **common,
            }
        )
    return in_maps


def kernel(q, k, v, Wq, bq, Wk, bk, Wv, bv, Wfc, bfc, ln_gamma, ln_beta):
    if "run" not in _CACHE:
        _CACHE["run"] = _make_runner(_build_program())
    in_maps = _shard_inputs(q, k, v, Wq, bq, Wk, bk, Wv, bv, Wfc, bfc,
                            ln_gamma, ln_beta)
    results = _CACHE["run"](in_maps)
    attn_flat = np.empty((H * B, S, S), np.float32)
    y = np.empty((B, S, D), np.float32)
    for c in range(N_CORES):
        b, qi = c // 4, c % 4
        rows = slice(SQ * qi, SQ * (qi + 1))
        for h in range(H):
            # device writes [sk, sq]; transpose back during unshard
            attn_flat[h * B + b, rows, :] = results[c]["attn_out"][h].T
        y[b, rows, :] = results[c]["y_out"]
    return (y, attn_flat)


# revision 20
# speedup vs baseline: 1.9432x; 1.1143x over previous
"""Trainium2 Bass kernel for MultiHeadAttention (B=2, S=2048, D=512, H=8).

Sharding: 8 cores; core c handles batch b=c//4, query-row quarter qi=c%4
(512 rows), all 8 heads. No collectives: fc + LayerNorm complete locally.

Per-core program (all fp32):
  phase 0: load weights/inputs, PE-transpose (identity matmul) to get
           WqT/WkT/WvT/WfcT, kT, vT, qsT; project qhT/khT [dk, S] and
           vh [S, D]; biases folded in (ACT Identity bias / K=1 ones-matmul).
  per head: scores = qhT_h.T @ khT_h (PE, K=64) -> PSUM; exp fused with
           1/sqrt(D) scale + rowsum on ACT (PSUM->SBUF); normalize with
           DVE tensor_scalar (per-partition 1/rowsum); DMA P out;
           PE-transpose P -> PT; AV: outT_h += vh_chunk.T @ PT_chunk.
  tail:    fc = sum_m outT[m].T @ WfcT[m] + bfc (ones-matmul) + residual,
           LayerNorm via bn_stats/bn_aggr, DMA y out.

_build_program(loop_n=R) wraps the whole body in a hardware For_i loop --
used by test.py to measure steady-state per-iteration device time.
"""

import numpy as np

B, S, D, H, DK = 2, 2048, 512, 8, 64
SQ = 512  # query rows per core
N_CORES = 8
TEMP = float(np.sqrt(D))
EPS = 1e-5

_CACHE = {}


def _build_program(loop_n=None, r32=True):
    import contextlib

    import concourse.bass as bass
    import concourse.tile as tile
    from concourse import bacc, mybir
    from concourse.masks import make_identity

    F32 = mybir.dt.float32
    AF = mybir.ActivationFunctionType

    # float32r: single-pass PE fp32 (1 cycle/row at N>=256 vs 4 cycles/row
    # for exact fp32). Tiles feeding matmuls are declared float32r so their
    # producing copy/activation rounds on write (BIR verifier requirement).
    MM = mybir.dt.float32r if r32 else F32

    def R(ap):
        return ap

    nc = bacc.Bacc("TRN2", target_bir_lowering=False, debug=False)

    # --- I/O ---
    qs_d = nc.dram_tensor("qs", [SQ, D], F32, kind="ExternalInput").ap()
    k_d = nc.dram_tensor("k", [S, D], F32, kind="ExternalInput").ap()
    v_d = nc.dram_tensor("v", [S, D], F32, kind="ExternalInput").ap()
    w_d = {
        n: nc.dram_tensor(n, [D, D], F32, kind="ExternalInput").ap()
        for n in ("wq", "wk", "wv", "wfc")
    }
    b_d = {
        n: nc.dram_tensor(n, [D], F32, kind="ExternalInput").ap()
        for n in ("bq", "bk", "bv", "bfc")
    }
    gamma_d = nc.dram_tensor("gamma", [D], F32, kind="ExternalInput").ap()
    beta_d = nc.dram_tensor("beta", [D], F32, kind="ExternalInput").ap()
    # transposed per-head layout [sk, sq]; host gather transposes back
    attn_d = nc.dram_tensor("attn_out", [H, S, SQ], F32, kind="ExternalOutput").ap()
    y_d = nc.dram_tensor("y_out", [SQ, D], F32, kind="ExternalOutput").ap()

    NSK = S // 128  # 16 sk row-chunks
    ND = D // 128  # 4 feature chunks
    NSQ = SQ // 128  # 4 sq sub-chunks

    with tile.TileContext(nc) as tc, contextlib.ExitStack() as ctx:
        const = ctx.enter_context(tc.tile_pool(name="const", bufs=1))
        persist = ctx.enter_context(tc.tile_pool(name="persist", bufs=1))

        ident = const.tile([128, 128], F32, name="ident")
        make_identity(nc, ident)
        ones_f32 = const.tile([128, 128], F32, name="ones_f32")
        nc.vector.memset(ones_f32, 1.0)
        ones1 = const.tile([1, 128], MM, name="ones1")
        nc.vector.tensor_copy(ones1, ones_f32[0:1, :])
        ones_col = const.tile([128, 1], MM, name="ones_col")
        nc.vector.tensor_copy(ones_col, ones_f32[:, 0:1])
        eps_t = const.tile([128, 1], F32, name="eps_t")
        nc.vector.memset(eps_t, EPS)
        # per-partition bias columns: [128, ND] col m = b[128m:128(m+1)]
        bq_col = const.tile([128, ND], F32, name="bq_col")
        nc.sync.dma_start(out=bq_col, in_=b_d["bq"].rearrange("(m p) -> p m", p=128))
        bk_col = const.tile([128, ND], F32, name="bk_col")
        nc.sync.dma_start(out=bk_col, in_=b_d["bk"].rearrange("(m p) -> p m", p=128))
        # row vectors for ones-matmul bias adds
        bv_row = const.tile([1, D], MM, name="bv_row")
        nc.sync.dma_start(out=bv_row, in_=b_d["bv"].rearrange("(o d) -> o d", o=1).bitcast(MM))
        bfc_row = const.tile([1, D], MM, name="bfc_row")
        nc.sync.dma_start(out=bfc_row, in_=b_d["bfc"].rearrange("(o d) -> o d", o=1).bitcast(MM))

        def bcast_rows(ap_1d):
            return bass.AP(
                tensor=ap_1d.tensor, offset=ap_1d.offset,
                ap=[[0, 128]] + [list(d) for d in ap_1d.ap],
            )

        gamma_bc = const.tile([128, D], F32, name="gamma_bc")
        nc.sync.dma_start(out=gamma_bc, in_=bcast_rows(gamma_d))
        beta_bc = const.tile([128, D], F32, name="beta_bc")
        nc.sync.dma_start(out=beta_bc, in_=bcast_rows(beta_d))

        def emit_iteration():
            # per-iteration persistent activations (slot-rotated, bufs=1)
            qs_nat = persist.tile([128, NSQ, D], F32, tag="qs_nat", name="qs_nat")
            nc.sync.dma_start(out=qs_nat, in_=qs_d.rearrange("(t p) d -> p t d", p=128))
            khT = [persist.tile([128, S], MM, tag=f"khT{m}", name=f"khT{m}")
                   for m in range(ND)]
            qhT = [persist.tile([128, SQ], MM, tag=f"qhT{m}", name=f"qhT{m}")
                   for m in range(ND)]
            vh = persist.tile([128, NSK, D], MM, tag="vh", name="vh")
            wfcT = [persist.tile([128, D], MM, tag=f"wfcT{m}", name=f"wfcT{m}")
                    for m in range(ND)]
            outT = [persist.tile([128, SQ], MM, tag=f"outT{m}", name=f"outT{m}")
                    for m in range(ND)]

            # ---------------- phase 0: transposes + projections ----------------
            with tc.tile_pool(name="ph0", bufs=1) as ph0, \
                 tc.tile_pool(name="ph0s", bufs=3) as ph0s, \
                 tc.tile_pool(name="ps0", bufs=2, space="PSUM") as ps0:

                def transpose_128xD_rows(dst_tiles, src_getter, n_row_chunks,
                                         copy_eng):
                    # group-blocked so each source chunk's lifetime is one group
                    for g in range((n_row_chunks + 3) // 4):
                        cnt = min(4, n_row_chunks - 4 * g)
                        srcs = [src_getter(4 * g + u) for u in range(cnt)]
                        for j in range(ND):
                            pt = ps0.tile([128, 512], F32, tag="tr",
                                          name=f"ptr_{j}_{g}")
                            for u in range(cnt):
                                nc.tensor.transpose(
                                    pt[:, 128 * u : 128 * (u + 1)],
                                    srcs[u][:, 128 * j : 128 * (j + 1)],
                                    ident,
                                )
                            if copy_eng == "act":
                                nc.scalar.copy(
                                    dst_tiles[j][:, 512 * g : 512 * g + 128 * cnt],
                                    pt[:, : 128 * cnt],
                                )
                            else:
                                nc.vector.tensor_copy(
                                    dst_tiles[j][:, 512 * g : 512 * g + 128 * cnt],
                                    pt[:, : 128 * cnt],
                                )

                # weights: load natural, transpose
                wT = {}
                for wi, wn in enumerate(("wq", "wk", "wv", "wfc")):
                    w_nat = ph0.tile([128, ND, D], F32, tag=f"wnat{wi % 2}",
                                     name=f"wnat_{wn}")
                    nc.sync.dma_start(
                        out=w_nat, in_=w_d[wn].rearrange("(t p) d -> p t d", p=128)
                    )
                    if wn == "wfc":
                        dst = wfcT
                    else:
                        dst = [ph0.tile([128, D], MM, tag=f"{wn}T{m}",
                                        name=f"{wn}T{m}") for m in range(ND)]
                        wT[wn] = dst
                    transpose_128xD_rows(dst, lambda i: w_nat[:, i, :], ND,
                                         "act" if wi % 2 else "dve")

                # kT / vT, then projections; k and v processed sequentially
                for name, src_d, n_rows in (("k", k_d, NSK), ("v", v_d, NSK)):
                    xT = [ph0.tile([128, S], MM, tag=f"xT{m}", name=f"{name}T{m}")
                          for m in range(ND)]

                    def load_chunk(i, _src=src_d, _name=name):
                        t = ph0s.tile([128, D], F32, tag="ld", bufs=8,
                                      name=f"{_name}n_{i}")
                        nc.sync.dma_start(out=t, in_=_src[128 * i : 128 * (i + 1), :])
                        return t

                    transpose_128xD_rows(xT, load_chunk, n_rows, "dve")

                    if name == "k":
                        for m in range(ND):
                            for n in range(S // 512):
                                pp = ps0.tile([128, 512], F32, tag="proj",
                                              name=f"pk_{m}_{n}")
                                for i in range(ND):
                                    nc.tensor.matmul(
                                        pp,
                                        R(wT["wk"][i][:, 128 * m : 128 * (m + 1)]),
                                        R(xT[i][:, 512 * n : 512 * (n + 1)]),
                                        start=(i == 0),
                                        stop=(i == ND - 1),
                                    )
                                nc.scalar.activation(
                                    khT[m][:, 512 * n : 512 * (n + 1)], pp,
                                    AF.Identity, bias=bk_col[:, m : m + 1],
                                )
                    else:
                        for p in range(NSK):
                            pp = ps0.tile([128, 512], F32, tag="proj",
                                          name=f"pv_{p}")
                            for i in range(ND):
                                nc.tensor.matmul(
                                    pp,
                                    R(xT[i][:, 128 * p : 128 * (p + 1)]),
                                    R(wT["wv"][i]),
                                    start=(i == 0),
                                    stop=False,
                                )
                            nc.tensor.matmul(pp, R(ones1), R(bv_row), start=False,
                                             stop=True)
                            nc.vector.tensor_copy(vh[:, p, :], pp)

                # qsT + qhT projection
                qsT = [ph0.tile([128, SQ], MM, tag=f"xT{m}", name=f"qsT{m}")
                       for m in range(ND)]
                transpose_128xD_rows(qsT, lambda i: qs_nat[:, i, :], NSQ, "dve")
                for m in range(ND):
                    pp = ps0.tile([128, 512], F32, tag="proj", name=f"pq_{m}")
                    for i in range(ND):
                        nc.tensor.matmul(
                            pp,
                            R(wT["wq"][i][:, 128 * m : 128 * (m + 1)]),
                            R(qsT[i]),
                            start=(i == 0),
                            stop=(i == ND - 1),
                        )
                    nc.scalar.activation(
                        qhT[m], pp, AF.Identity, bias=bq_col[:, m : m + 1],
                    )

            # ---------------- attention per head (transposed flow) ----------
            # sT[sk, sq] = khT_h.T @ qhT_h; exp fused in the PSUM->SBUF copy;
            # colsum via ones-matmul on PE; normalize P~T in place; write
            # attn transposed; AV consumes P~T chunks directly.
            with tc.tile_pool(name="expt_pool", bufs=2) as expt_pool, \
                 tc.tile_pool(name="small", bufs=4) as small, \
                 tc.tile_pool(name="psA", bufs=2, space="PSUM") as psA:

                expTs = {}

                def emit_av(h):
                    # AV: outT_h [64, SQ] accumulated over sk chunks
                    hp, hr = h // 2, h % 2
                    expT = expTs.pop(h)
                    pav = psA.tile([64, 512], F32, tag="av", bufs=2,
                                   name=f"pav_{h}")
                    for skc in range(NSK):
                        nc.tensor.matmul(
                            pav,
                            R(vh[:, skc, 64 * h : 64 * h + 64]),
                            R(expT[:, skc, :]),
                            start=(skc == 0),
                            stop=(skc == NSK - 1),
                        )
                    nc.vector.tensor_copy(outT[hp][64 * hr : 64 * hr + 64, :], pav)

                for h in range(H):
                    hp, hr = h // 2, h % 2
                    qhT_h = qhT[hp][64 * hr : 64 * hr + 64, :]
                    khT_h = khT[hp][64 * hr : 64 * hr + 64, :]

                    expT = expt_pool.tile([128, NSK, SQ], MM, tag="expt",
                                          name=f"expT_{h}")
                    expTs[h] = expT
                    for skc in range(NSK):
                        psc = psA.tile([128, SQ], F32, tag="sc", bufs=3,
                                       name=f"sc_{h}_{skc}")
                        nc.tensor.matmul(
                            psc,
                            R(khT_h[:, 128 * skc : 128 * (skc + 1)]),
                            R(qhT_h),
                            start=True, stop=True,
                        )
                        nc.scalar.activation(
                            expT[:, skc, :], psc, AF.Exp, scale=1.0 / TEMP,
                        )
                    pcs = psA.tile([1, SQ], F32, tag="cs", bufs=2,
                                   name=f"pcs_{h}")
                    for skc in range(NSK):
                        nc.tensor.matmul(
                            pcs, R(ones_col), R(expT[:, skc, :]),
                            start=(skc == 0), stop=(skc == NSK - 1),
                        )
                    # previous head's AV runs on PE while this head's
                    # normalize chain (DVE/POOL) completes
                    if h > 0:
                        emit_av(h - 1)
                    rs_sb = small.tile([1, SQ], F32, tag="rs", name=f"rs_{h}")
                    nc.vector.tensor_copy(rs_sb, pcs)
                    inv_r = small.tile([1, SQ], F32, tag="invr", name=f"inv_{h}")
                    rscr = small.tile([1, SQ], F32, tag="rscr", name=f"rscr_{h}")
                    nc.vector.reciprocal_approx_accurate(inv_r, rs_sb, rscr)
                    inv_bc = small.tile([128, SQ], F32, tag="invbc",
                                        name=f"invbc_{h}")
                    nc.gpsimd.partition_broadcast(inv_bc, inv_r)
                    # normalize in place; inv broadcast along the skc dim
                    inv_bc3 = bass.AP(
                        tensor=inv_bc.tensor, offset=inv_bc.offset,
                        ap=[list(inv_bc.ap[0]), [0, NSK], list(inv_bc.ap[1])],
                    )
                    nc.vector.tensor_mul(expT, expT, inv_bc3)
                    nc.sync.dma_start(
                        out=attn_d[h].rearrange("(t p) q -> p t q", p=128),
                        in_=expT.bitcast(F32),
                    )
                emit_av(H - 1)

            # ---------------- fc + residual + LayerNorm ----------------
            with tc.tile_pool(name="tail", bufs=4) as tail, \
                 tc.tile_pool(name="psY", bufs=2, space="PSUM") as psY:
                for s in range(NSQ):
                    py = psY.tile([128, 512], F32, tag="y", name=f"py_{s}")
                    for m in range(ND):
                        nc.tensor.matmul(
                            py,
                            R(outT[m][:, 128 * s : 128 * (s + 1)]),
                            R(wfcT[m]),
                            start=(m == 0),
                            stop=False,
                        )
                    nc.tensor.matmul(py, R(ones1), R(bfc_row), start=False,
                                     stop=True)
                    x_sb = tail.tile([128, D], F32, tag="x", name=f"x_{s}")
                    nc.vector.tensor_add(x_sb, py, qs_nat[:, s, :])
                    stats = tail.tile([128, 6], F32, tag="st", name=f"st_{s}")
                    nc.vector.bn_stats(out=stats, in_=x_sb)
                    mv = tail.tile([128, 2], F32, tag="mv", name=f"mv_{s}")
                    nc.vector.bn_aggr(out=mv, in_=stats)
                    rstd = tail.tile([128, 1], F32, tag="rstd", name=f"rstd_{s}")
                    nc.scalar.activation(rstd, mv[:, 1:2], AF.Sqrt, bias=eps_t)
                    nc.vector.reciprocal(rstd, rstd)
                    xn = tail.tile([128, D], F32, tag="xn", name=f"xn_{s}")
                    nc.vector.tensor_scalar(
                        xn, x_sb, mv[:, 0:1], rstd,
                        op0=mybir.AluOpType.subtract, op1=mybir.AluOpType.mult,
                    )
                    nc.vector.tensor_mul(xn, xn, gamma_bc)
                    nc.vector.tensor_add(xn, xn, beta_bc)
                    nc.sync.dma_start(out=y_d[128 * s : 128 * (s + 1), :], in_=xn)

        if loop_n:
            with tc.For_i(0, loop_n, 1,
                          hint_engines=(mybir.EngineType.PE,
                                        mybir.EngineType.Activation,
                                        mybir.EngineType.DVE)):
                emit_iteration()
        else:
            emit_iteration()

    nc.compile()
    return nc


def _make_runner(nc):
    """Cached jitted SPMD runner for a compiled Bass program (mirrors
    concourse.bass2jax.run_bass_via_pjrt's multi-core branch)."""
    import jax
    from jax.experimental.shard_map import shard_map
    from jax.sharding import Mesh, PartitionSpec
    from concourse import bass2jax, mybir as _mybir

    bass2jax.install_neuronx_cc_hook()

    partition_name = nc.partition_id_tensor.name if nc.partition_id_tensor else None
    in_names, out_names, out_avals, zero_outs = [], [], [], []
    for alloc in nc.m.functions[0].allocations:
        if not isinstance(alloc, _mybir.MemoryLocationSet):
            continue
        name = alloc.memorylocations[0].name
        if alloc.kind == "ExternalInput":
            if name != partition_name:
                in_names.append(name)
        elif alloc.kind == "ExternalOutput":
            shape = tuple(alloc.tensor_shape)
            dtype = _mybir.dt.np(alloc.dtype)
            out_names.append(name)
            out_avals.append(jax.core.ShapedArray(shape, dtype))
            zero_outs.append(np.zeros(shape, dtype))
    n_params = len(in_names)
    n_outs = len(out_avals)
    all_in_names = in_names + out_names + (
        [partition_name] if partition_name else []
    )
    donate = tuple(range(n_params, n_params + n_outs))

    def _body(*args):
        operands = list(args)
        if partition_name is not None:
            operands.append(bass2jax.partition_id_tensor())
        outs = bass2jax._bass_exec_p.bind(
            *operands,
            out_avals=tuple(out_avals),
            in_names=tuple(all_in_names),
            out_names=tuple(out_names),
            lowering_input_output_aliases=(),
            sim_require_finite=True,
            sim_require_nnan=True,
            nc=nc,
        )
        return tuple(outs)

    devices = jax.devices()[:N_CORES]
    mesh = Mesh(np.asarray(devices), ("core",))
    in_specs = (PartitionSpec("core"),) * (n_params + n_outs)
    out_specs = (PartitionSpec("core"),) * n_outs
    sharded = jax.jit(
        shard_map(_body, mesh=mesh, in_specs=in_specs, out_specs=out_specs,
                  check_rep=False),
        donate_argnums=donate,
        keep_unused=True,
    )

    def run(in_maps):
        per_core = [[np.asarray(m[n]) for n in in_names] for m in in_maps]
        concat_in = [
            np.concatenate([per_core[c][i] for c in range(N_CORES)], axis=0)
            for i in range(n_params)
        ]
        concat_zeros = [
            np.zeros((N_CORES * z.shape[0], *z.shape[1:]), z.dtype)
            for z in zero_outs
        ]
        out_arrs = sharded(*concat_in, *concat_zeros)
        return [
            {
                n: np.asarray(out_arrs[i]).reshape(N_CORES, *out_avals[i].shape)[c]
                for i, n in enumerate(out_names)
            }
            for c in range(N_CORES)
        ]

    return run


def _make_bench_fn(nc, in_maps):
    """Timing-only runner: inputs (and dummy zero outputs) are device_put
    once; no donation, outputs never fetched. Per-call cost = dispatch +
    device execution."""
    import jax
    from jax.experimental.shard_map import shard_map
    from jax.sharding import Mesh, NamedSharding, PartitionSpec
    from concourse import bass2jax, mybir as _mybir

    bass2jax.install_neuronx_cc_hook()

    partition_name = nc.partition_id_tensor.name if nc.partition_id_tensor else None
    in_names, out_names, out_avals, zero_outs = [], [], [], []
    for alloc in nc.m.functions[0].allocations:
        if not isinstance(alloc, _mybir.MemoryLocationSet):
            continue
        name = alloc.memorylocations[0].name
        if alloc.kind == "ExternalInput":
            if name != partition_name:
                in_names.append(name)
        elif alloc.kind == "ExternalOutput":
            shape = tuple(alloc.tensor_shape)
            dtype = _mybir.dt.np(alloc.dtype)
            out_names.append(name)
            out_avals.append(jax.core.ShapedArray(shape, dtype))
            zero_outs.append(np.zeros(shape, dtype))
    n_params = len(in_names)
    all_in_names = in_names + out_names + (
        [partition_name] if partition_name else []
    )

    def _body(*args):
        operands = list(args)
        if partition_name is not None:
            operands.append(bass2jax.partition_id_tensor())
        outs = bass2jax._bass_exec_p.bind(
            *operands,
            out_avals=tuple(out_avals),
            in_names=tuple(all_in_names),
            out_names=tuple(out_names),
            lowering_input_output_aliases=(),
            sim_require_finite=True,
            sim_require_nnan=True,
            nc=nc,
        )
        return tuple(outs)

    devices = jax.devices()[:N_CORES]
    mesh = Mesh(np.asarray(devices), ("core",))
    nsh = NamedSharding(mesh, PartitionSpec("core"))
    in_specs = (PartitionSpec("core"),) * (n_params + len(out_avals))
    out_specs = (PartitionSpec("core"),) * len(out_avals)
    sharded = jax.jit(
        shard_map(_body, mesh=mesh, in_specs=in_specs, out_specs=out_specs,
                  check_rep=False),
        keep_unused=True,
    )

    per_core = [[np.asarray(m[n]) for n in in_names] for m in in_maps]
    dev_args = [
        jax.device_put(
            np.concatenate([per_core[c][i] for c in range(N_CORES)], axis=0), nsh
        )
        for i in range(n_params)
    ] + [
        jax.device_put(np.zeros((N_CORES * z.shape[0], *z.shape[1:]), z.dtype), nsh)
        for z in zero_outs
    ]

    def call_once():
        import time as _t
        t0 = _t.time()
        out = sharded(*dev_args)
        jax.block_until_ready(out)
        return _t.time() - t0

    return call_once


def _shard_inputs(q, k, v, Wq, bq, Wk, bk, Wv, bv, Wfc, bfc, ln_gamma, ln_beta):
    q = np.ascontiguousarray(np.asarray(q, dtype=np.float32))
    k = np.ascontiguousarray(np.asarray(k, dtype=np.float32))
    v = np.ascontiguousarray(np.asarray(v, dtype=np.float32))
    common = {
        "wq": np.ascontiguousarray(np.asarray(Wq, np.float32)),
        "wk": np.ascontiguousarray(np.asarray(Wk, np.float32)),
        "wv": np.ascontiguousarray(np.asarray(Wv, np.float32)),
        "wfc": np.ascontiguousarray(np.asarray(Wfc, np.float32)),
        "bq": np.ascontiguousarray(np.asarray(bq, np.float32)),
        "bk": np.ascontiguousarray(np.asarray(bk, np.float32)),
        "bv": np.ascontiguousarray(np.asarray(bv, np.float32)),
        "bfc": np.ascontiguousarray(np.asarray(bfc, np.float32)),
        "gamma": np.ascontiguousarray(np.asarray(ln_gamma, np.float32)),
        "beta": np.ascontiguousarray(np.asarray(ln_beta, np.float32)),
    }
    in_maps = []
    for c in range(N_CORES):
        b, qi = c // 4, c % 4
        rows = slice(SQ * qi, SQ * (qi + 1))
        in_maps.append(
            {
                "qs": np.ascontiguousarray(q[b, rows, :]),
                "k": np.ascontiguousarray(k[b]),
                "v": np.ascontiguousarray(v[b]),
                **common,
            }
        )
    return in_maps


def kernel(q, k, v, Wq, bq, Wk, bk, Wv, bv, Wfc, bfc, ln_gamma, ln_beta):
    if "run" not in _CACHE:
        _CACHE["run"] = _make_runner(_build_program())
    in_maps = _shard_inputs(q, k, v, Wq, bq, Wk, bk, Wv, bv, Wfc, bfc,
                            ln_gamma, ln_beta)
    results = _CACHE["run"](in_maps)
    attn_flat = np.empty((H * B, S, S), np.float32)
    y = np.empty((B, S, D), np.float32)
    for c in range(N_CORES):
        b, qi = c // 4, c % 4
        rows = slice(SQ * qi, SQ * (qi + 1))
        for h in range(H):
            # device writes [sk, sq]; transpose back during unshard
            attn_flat[h * B + b, rows, :] = results[c]["attn_out"][h].T
        y[b, rows, :] = results[c]["y_out"]
    return (y, attn_flat)
